# revision 17
# baseline (speedup 1.0000x reference)
"""Trainium2 Bass kernel for nn_CustomConv2d: 3x3 conv, B=16, Cin=Cout=128, H=W=64.

Strategy:
  - Data-parallel over batch: 8 NeuronCores x 2 images each; the (128,128,9)
    weight is replicated (host pre-transposes it to [cin, k, cout] so tap k is
    a contiguous [cin, cout] stationary-operand slice).
  - Per image the feature map lives in SBUF as a 66x66 zero-padded plane
    (host-prepadded, so every DMA is fully contiguous):
      row r in [-1,64], col c in [-1,64] at offset (r+1)*66 + (c+1).
  - Conv = 9 accumulating PE matmuls per 8-row output block (contraction over
    Cin=128 on the partition dim).  Tap (dy,dx) reads the 2D window
    [[66,8],[1,64]] at offset (y0+dy)*66 + dx; the padding zeros make every
    tap exact, so there is no edge fixup of any kind.
  - All matmul operands are bf16 (host RNE-rounds fp32 -> bf16): same 1
    cycle/row PE stream rate as fp32r, but half the input DMA bytes and the
    compiler's fast-weight-load (FWL) path for LDWEIGHTS.  PSUM accumulates
    fp32; outputs are stored fp32.  Measured rel err vs f64 ref ~2.5e-3.
  - DMA plan (descriptor-gen is ~0.65us per DMA on the issuing engine, so
    big chunks, split across engines): scalar streams img0 (small first
    chunk to unblock block 0 ASAP), sync streams weights then img1,
    gpsimd memsets the warmup tile.  Output stores are paired (2 blocks =
    4KB/partition per DMA) and split scalar(img0)/sync(img1); the final
    block is split in halves across scalar+sync to shorten the drain.
  - A short bf16 dummy-matmul warmup starts PE activity right after the
    framework preamble (~7us) so the HAM clock gate (3.4us busy window,
    1.2->2.4 GHz) releases before the bulk of the real matmul stream.
"""

import numpy as np
import ml_dtypes

import concourse.bass as bass  # noqa: F401  (registers bass types)
import concourse.tile as tile
import concourse.mybir as mybir
from concourse import bacc, bass_utils

F32 = mybir.dt.float32
BF16 = mybir.dt.bfloat16

B, CIN, COUT, KK, H, W = 16, 128, 128, 3, 64, 64
NCORES = 8
BPC = B // NCORES  # batches per core
HW = H * W         # 4096
PW = W + 2         # padded row length (66)
PH = H + 2         # padded rows (66)
XLEN = PH * PW     # 4356
ROWBLK = 8         # output rows per PSUM block (8*64=512 = one fp32 PSUM bank)
NBLK = H // ROWBLK

TRACE = False      # set True to capture an NTFF profile (fills LAST_EXEC_NS)
LAST_EXEC_NS = None

_CACHE = {}


def _build():
    nc = bacc.Bacc("TRN2", target_bir_lowering=False, debug=False, num_devices=NCORES)
    x_d = nc.dram_tensor("x", [BPC, CIN, XLEN], BF16, kind="ExternalInput").ap()
    w_d = nc.dram_tensor("w", [CIN, KK * KK * COUT], BF16, kind="ExternalInput").ap()
    o_d = nc.dram_tensor("o", [BPC, COUT, HW], F32, kind="ExternalOutput").ap()

    with tile.TileContext(nc) as tc:
        with (
            tc.tile_pool(name="wt", bufs=1) as wtp,
            tc.tile_pool(name="xin", bufs=2) as xp,
            tc.tile_pool(name="ps", bufs=4, space="PSUM") as pp,
            tc.tile_pool(name="ot", bufs=5) as op,
            tc.tile_pool(name="warm", bufs=1) as wmp,
            tc.tile_pool(name="warmps", bufs=1, space="PSUM") as wpp,
        ):
            wt = wtp.tile([CIN, KK * KK * COUT], BF16)
            xins = [
                xp.tile([CIN, XLEN], BF16, tag="xin", name=f"xin{i}")
                for i in range(BPC)
            ]

            # --- input DMAs.  A 128-partition load costs one packet per
            # partition on its queue (~10-20ns each after a ~1.6us
            # desc-gen+ring latency), and rates are per-queue — so the two
            # head-critical pieces (all 9 weight taps; img0 rows 0-9) go
            # first on the two fast HWDGE queues (sync / scalar), with later
            # chunks sized so each lands just before its consumer block.
            # The gpsimd software-DGE queue is slow to start (~+1us), so it
            # only carries img1 (needed from ~25us).
            # sync: w taps 0-5 ALONE (gates the first matmul; a queue's
            # semaphore packets lag behind any later DMA's data packets on
            # the same ring, so nothing else may queue behind it).  Sync
            # then carries all output stores — no input contention.
            nc.sync.dma_start(wt[:, : 6 * COUT], w_d[:, : 6 * COUT])
            # scalar: img0 rows in consumption order, sized so each chunk
            # lands just before the block that needs it.
            for r0, r1 in ((0, 10), (10, 22), (22, 34), (34, PH)):
                nc.scalar.dma_start(
                    xins[0][:, PW * r0 : PW * r1], x_d[0][:, PW * r0 : PW * r1]
                )
            # gpsimd (slow-start software-DGE ring): w taps 6-8 (needed at
            # first-MM + 1.3us) then img1, first consumed at ~26us.
            nc.gpsimd.dma_start(wt[:, 6 * COUT :], w_d[:, 6 * COUT :])
            for r0, r1 in ((0, 34), (34, PH)):
                nc.gpsimd.dma_start(
                    xins[1][:, PW * r0 : PW * r1], x_d[1][:, PW * r0 : PW * r1]
                )

            # --- PE warm-up: HAM releases the clock gate only after ~3.4us of
            # sustained array activity; dummy bf16 matmuls on a zeroed tile
            # bridge from the framework preamble (~7.5us) to first data
            # (~10.3us) so most of the real stream runs at 2.4 GHz.
            wz = wmp.tile([CIN, 4 * COUT], BF16)
            nc.vector.memset(wz[:], 0.0)
            wps = wpp.tile([COUT, 4 * COUT], F32)
            for _ in range(6):
                nc.tensor.matmul(wps[:], wz[:, :COUT], wz[:], start=True, stop=True)

            # --- main conv: per image, per 8-row block, 9 accumulating MMs.
            for lb in range(BPC):
                xrf = xins[lb][:].rearrange("p (r c) -> p r c", c=PW)  # [128,66,66]
                ot = None
                for yb in range(NBLK):
                    y0 = yb * ROWBLK
                    ps = pp.tile([COUT, ROWBLK * W], F32)
                    first = True
                    for dy in range(KK):
                        for dx in range(KK):
                            nc.tensor.matmul(
                                ps[:],
                                wt[:, (dy * KK + dx) * COUT : (dy * KK + dx + 1) * COUT],
                                xrf[:, y0 + dy : y0 + dy + ROWBLK, dx : dx + W],
                                start=first,
                                stop=(dy == KK - 1 and dx == KK - 1),
                            )
                            first = False
                    # all paired stores on sync (idle after the two weight
                    # loads; DRAM writes are cheap when the ring is exclusive).
                    eng = nc.sync
                    final_img = lb == BPC - 1
                    if final_img and yb == NBLK - 1:
                        # final block: quarter copies split across the two
                        # PSUM-capable engines (vector + scalar), stores
                        # spread over three idle queues, so the kernel-exit
                        # drain starts as soon after the last matmul as
                        # possible.
                        ot = op.tile([COUT, ROWBLK * W], F32, name="ot_last")
                        qw = ROWBLK * W // 4
                        for q, ceng, e in (
                            (0, nc.vector, nc.sync),
                            (1, nc.scalar, nc.gpsimd),
                            (2, nc.vector, nc.scalar),
                            (3, nc.scalar, nc.sync),
                        ):
                            sl = slice(q * qw, (q + 1) * qw)
                            if ceng is nc.scalar:
                                ceng.copy(ot[:, sl], ps[:, sl])
                            else:
                                ceng.tensor_copy(ot[:, sl], ps[:, sl])
                            e.dma_start(
                                o_d[lb][:, W * y0 + q * qw : W * y0 + (q + 1) * qw],
                                ot[:, sl],
                            )
                    elif final_img and yb == NBLK - 2:
                        # penultimate block of the last image: single store so
                        # the final block isn't held behind a pair.
                        ot = op.tile([COUT, ROWBLK * W], F32, name="ot_single")
                        nc.vector.tensor_copy(ot[:], ps[:])
                        eng.dma_start(o_d[lb][:, W * y0 : W * (y0 + ROWBLK)], ot[:])
                    elif yb % 2 == 0:
                        # even block: stage into the low half of a paired tile
                        ot = op.tile([COUT, 2 * ROWBLK * W], F32, name="ot_pair")
                        nc.vector.tensor_copy(ot[:, : ROWBLK * W], ps[:])
                    else:
                        # odd block: high half, then one 4KB/partition store
                        nc.vector.tensor_copy(ot[:, ROWBLK * W :], ps[:])
                        eng.dma_start(
                            o_d[lb][:, W * (y0 - ROWBLK) : W * (y0 + ROWBLK)], ot[:]
                        )
    nc.compile()
    return nc


def _get_nc():
    if "nc" not in _CACHE:
        _CACHE["nc"] = _build()
    return _CACHE["nc"]


def kernel(x, weights):
    """x: [16,128,64,64] f32; weights: [128,128,9] f32 -> [2048,64,64] f32."""
    global LAST_EXEC_NS
    x = np.asarray(x, dtype=np.float32)
    w = np.asarray(weights, dtype=np.float32)
    # [cout, cin, k] -> [cin, k, cout] so tap k is a contiguous lhsT slice
    wT = np.ascontiguousarray(w.transpose(1, 2, 0)).reshape(CIN, KK * KK * COUT)
    wT = wT.astype(ml_dtypes.bfloat16)  # RNE
    xpad = np.zeros((B, CIN, PH, PW), ml_dtypes.bfloat16)
    xpad[:, :, 1 : H + 1, 1 : W + 1] = x.astype(ml_dtypes.bfloat16)
    xpad = xpad.reshape(B, CIN, XLEN)

    nc = _get_nc()
    xr = xpad.reshape(NCORES, BPC, CIN, XLEN)
    in_maps = [{"x": np.ascontiguousarray(xr[c]), "w": wT} for c in range(NCORES)]

    res = bass_utils.run_bass_kernel_spmd(
        nc, in_maps, core_ids=list(range(NCORES)), trace=TRACE
    )
    LAST_EXEC_NS = res.exec_time_ns

    arr = np.stack([res.results[c]["o"] for c in range(NCORES)])  # [8, 2, 128, 4096]
    # out[cout*B + b] = conv[b, cout], with b = core*BPC + lb
    arr = arr.transpose(2, 0, 1, 3).reshape(COUT, B, H, W)
    return np.ascontiguousarray(arr.reshape(COUT * B, H, W))


# revision 19
# speedup vs baseline: 1.0342x; 1.0342x over previous
"""Trainium2 Bass kernel for nn_CustomConv2d: 3x3 conv, B=16, Cin=Cout=128, H=W=64.

Strategy:
  - Data-parallel over batch: 8 NeuronCores x 2 images each; the (128,128,9)
    weight is replicated (host pre-transposes it to [cin, k, cout] so tap k is
    a contiguous [cin, cout] stationary-operand slice).
  - Per image the feature map lives in SBUF as a 66x66 zero-padded plane
    (host-prepadded, so every DMA is fully contiguous):
      row r in [-1,64], col c in [-1,64] at offset (r+1)*66 + (c+1).
  - Conv = 9 accumulating PE matmuls per 8-row output block (contraction over
    Cin=128 on the partition dim).  Tap (dy,dx) reads the 2D window
    [[66,8],[1,64]] at offset (y0+dy)*66 + dx; the padding zeros make every
    tap exact, so there is no edge fixup of any kind.
  - All matmul operands are bf16 (host RNE-rounds fp32 -> bf16): same 1
    cycle/row PE stream rate as fp32r, but half the input DMA bytes and the
    compiler's fast-weight-load (FWL) path for LDWEIGHTS.  PSUM accumulates
    fp32; outputs are stored fp32.  Measured rel err vs f64 ref ~2.5e-3.
  - DMA plan (descriptor-gen is ~0.65us per DMA on the issuing engine, so
    big chunks, split across engines): scalar streams img0 (small first
    chunk to unblock block 0 ASAP), sync streams weights then img1,
    gpsimd memsets the warmup tile.  Output stores are paired (2 blocks =
    4KB/partition per DMA) and split scalar(img0)/sync(img1); the final
    block is split in halves across scalar+sync to shorten the drain.
  - A short bf16 dummy-matmul warmup starts PE activity right after the
    framework preamble (~7us) so the HAM clock gate (3.4us busy window,
    1.2->2.4 GHz) releases before the bulk of the real matmul stream.
"""

import numpy as np
import ml_dtypes

import concourse.bass as bass  # noqa: F401  (registers bass types)
import concourse.tile as tile
import concourse.mybir as mybir
from concourse import bacc, bass_utils

F32 = mybir.dt.float32
BF16 = mybir.dt.bfloat16

B, CIN, COUT, KK, H, W = 16, 128, 128, 3, 64, 64
NCORES = 8
BPC = B // NCORES  # batches per core
HW = H * W         # 4096
PW = W + 2         # padded row length (66)
PH = H + 2         # padded rows (66)
XLEN = PH * PW     # 4356
ROWBLK = 8         # output rows per PSUM block (8*64=512 = one fp32 PSUM bank)
NBLK = H // ROWBLK

TRACE = False      # set True to capture an NTFF profile (fills LAST_EXEC_NS)
LAST_EXEC_NS = None

_CACHE = {}


def _build():
    nc = bacc.Bacc("TRN2", target_bir_lowering=False, debug=False, num_devices=NCORES)
    x_d = nc.dram_tensor("x", [BPC, CIN, XLEN], BF16, kind="ExternalInput").ap()
    w_d = nc.dram_tensor("w", [CIN, KK * KK * COUT], BF16, kind="ExternalInput").ap()
    o_d = nc.dram_tensor("o", [BPC, COUT, HW], F32, kind="ExternalOutput").ap()

    with tile.TileContext(nc) as tc:
        with (
            tc.tile_pool(name="wt", bufs=1) as wtp,
            tc.tile_pool(name="xin", bufs=2) as xp,
            tc.tile_pool(name="ps", bufs=4, space="PSUM") as pp,
            tc.tile_pool(name="ot", bufs=5) as op,
            tc.tile_pool(name="warm", bufs=1) as wmp,
            tc.tile_pool(name="warmps", bufs=1, space="PSUM") as wpp,
        ):
            wt = wtp.tile([CIN, KK * KK * COUT], BF16)
            xins = [
                xp.tile([CIN, XLEN], BF16, tag="xin", name=f"xin{i}")
                for i in range(BPC)
            ]

            # --- input DMAs.  A 128-partition load costs one packet per
            # partition on its queue (~10-20ns each after a ~1.6us
            # desc-gen+ring latency), and rates are per-queue — so the two
            # head-critical pieces (all 9 weight taps; img0 rows 0-9) go
            # first on the two fast HWDGE queues (sync / scalar), with later
            # chunks sized so each lands just before its consumer block.
            # The gpsimd software-DGE queue is slow to start (~+1us), so it
            # only carries img1 (needed from ~25us).
            # sync: w taps 0-5 ALONE (gates the first matmul; a queue's
            # semaphore packets lag behind any later DMA's data packets on
            # the same ring, so nothing else may queue behind it).  Sync
            # then carries all output stores — no input contention.
            nc.sync.dma_start(wt[:, : 6 * COUT], w_d[:, : 6 * COUT])
            # scalar: img0 rows 0-33 in consumption order (block k's rows
            # land just ahead of its matmuls), then img1 (needed from ~26us).
            for r0, r1 in ((0, 10), (10, 22), (22, 34)):
                nc.scalar.dma_start(
                    xins[0][:, PW * r0 : PW * r1], x_d[0][:, PW * r0 : PW * r1]
                )
            for r0, r1 in ((0, 34), (34, PH)):
                nc.scalar.dma_start(
                    xins[1][:, PW * r0 : PW * r1], x_d[1][:, PW * r0 : PW * r1]
                )
            # gpsimd (slow-start software-DGE ring): w taps 6-8 (needed at
            # first-MM + 1.3us) then img0 rows 34-65 (needed from ~18us).
            nc.gpsimd.dma_start(wt[:, 6 * COUT :], w_d[:, 6 * COUT :])
            nc.gpsimd.dma_start(
                xins[0][:, PW * 34 :], x_d[0][:, PW * 34 :]
            )

            # --- PE warm-up: HAM releases the clock gate only after ~3.4us of
            # sustained array activity; dummy bf16 matmuls on a zeroed tile
            # bridge from the framework preamble (~7.5us) to first data
            # (~10.3us) so most of the real stream runs at 2.4 GHz.
            wz = wmp.tile([CIN, 4 * COUT], BF16)
            nc.vector.memset(wz[:], 0.0)
            wps = wpp.tile([COUT, 4 * COUT], F32)
            for _ in range(6):
                nc.tensor.matmul(wps[:], wz[:, :COUT], wz[:], start=True, stop=True)

            # --- main conv: per image, per 8-row block, 9 accumulating MMs.
            for lb in range(BPC):
                xrf = xins[lb][:].rearrange("p (r c) -> p r c", c=PW)  # [128,66,66]
                ot = None
                for yb in range(NBLK):
                    y0 = yb * ROWBLK
                    ps = pp.tile([COUT, ROWBLK * W], F32)
                    first = True
                    for dy in range(KK):
                        for dx in range(KK):
                            nc.tensor.matmul(
                                ps[:],
                                wt[:, (dy * KK + dx) * COUT : (dy * KK + dx + 1) * COUT],
                                xrf[:, y0 + dy : y0 + dy + ROWBLK, dx : dx + W],
                                start=first,
                                stop=(dy == KK - 1 and dx == KK - 1),
                            )
                            first = False
                    # all paired stores on sync (idle after the two weight
                    # loads; DRAM writes are cheap when the ring is exclusive).
                    eng = nc.sync
                    final_img = lb == BPC - 1
                    if final_img and yb == NBLK - 1:
                        # final block: two independent copy->store chains with
                        # no cross-engine FIFO coupling — vector copy feeds a
                        # sync store, scalar copy feeds scalar's own store.
                        ot = op.tile([COUT, ROWBLK * W], F32, name="ot_last")
                        hw2 = ROWBLK * W // 2
                        nc.vector.tensor_copy(ot[:, :hw2], ps[:, :hw2])
                        nc.sync.dma_start(
                            o_d[lb][:, W * y0 : W * y0 + hw2], ot[:, :hw2]
                        )
                        nc.scalar.copy(ot[:, hw2:], ps[:, hw2:])
                        nc.scalar.dma_start(
                            o_d[lb][:, W * y0 + hw2 : W * y0 + 2 * hw2],
                            ot[:, hw2:],
                        )
                    elif final_img and yb == NBLK - 2:
                        # penultimate block of the last image: single store so
                        # the final block isn't held behind a pair.
                        ot = op.tile([COUT, ROWBLK * W], F32, name="ot_single")
                        nc.vector.tensor_copy(ot[:], ps[:])
                        eng.dma_start(o_d[lb][:, W * y0 : W * (y0 + ROWBLK)], ot[:])
                    elif yb % 2 == 0:
                        # even block: stage into the low half of a paired tile
                        ot = op.tile([COUT, 2 * ROWBLK * W], F32, name="ot_pair")
                        nc.vector.tensor_copy(ot[:, : ROWBLK * W], ps[:])
                    else:
                        # odd block: high half, then one 4KB/partition store
                        nc.vector.tensor_copy(ot[:, ROWBLK * W :], ps[:])
                        eng.dma_start(
                            o_d[lb][:, W * (y0 - ROWBLK) : W * (y0 + ROWBLK)], ot[:]
                        )
    nc.compile()
    return nc


def _get_nc():
    if "nc" not in _CACHE:
        _CACHE["nc"] = _build()
    return _CACHE["nc"]


def kernel(x, weights):
    """x: [16,128,64,64] f32; weights: [128,128,9] f32 -> [2048,64,64] f32."""
    global LAST_EXEC_NS
    x = np.asarray(x, dtype=np.float32)
    w = np.asarray(weights, dtype=np.float32)
    # [cout, cin, k] -> [cin, k, cout] so tap k is a contiguous lhsT slice
    wT = np.ascontiguousarray(w.transpose(1, 2, 0)).reshape(CIN, KK * KK * COUT)
    wT = wT.astype(ml_dtypes.bfloat16)  # RNE
    xpad = np.zeros((B, CIN, PH, PW), ml_dtypes.bfloat16)
    xpad[:, :, 1 : H + 1, 1 : W + 1] = x.astype(ml_dtypes.bfloat16)
    xpad = xpad.reshape(B, CIN, XLEN)

    nc = _get_nc()
    xr = xpad.reshape(NCORES, BPC, CIN, XLEN)
    in_maps = [{"x": np.ascontiguousarray(xr[c]), "w": wT} for c in range(NCORES)]

    res = bass_utils.run_bass_kernel_spmd(
        nc, in_maps, core_ids=list(range(NCORES)), trace=TRACE
    )
    LAST_EXEC_NS = res.exec_time_ns

    arr = np.stack([res.results[c]["o"] for c in range(NCORES)])  # [8, 2, 128, 4096]
    # out[cout*B + b] = conv[b, cout], with b = core*BPC + lb
    arr = arr.transpose(2, 0, 1, 3).reshape(COUT, B, H, W)
    return np.ascontiguousarray(arr.reshape(COUT * B, H, W))


# revision 20
# speedup vs baseline: 1.0655x; 1.0303x over previous
"""Trainium2 Bass kernel for nn_CustomConv2d: 3x3 conv, B=16, Cin=Cout=128, H=W=64.

Strategy:
  - Data-parallel over batch: 8 NeuronCores x 2 images each; the (128,128,9)
    weight is replicated (host pre-transposes it to [cin, k, cout] so tap k is
    a contiguous [cin, cout] stationary-operand slice).
  - Per image the feature map lives in SBUF as a 66x66 zero-padded plane
    (host-prepadded, so every DMA is fully contiguous):
      row r in [-1,64], col c in [-1,64] at offset (r+1)*66 + (c+1).
  - Conv = 9 accumulating PE matmuls per 8-row output block (contraction over
    Cin=128 on the partition dim).  Tap (dy,dx) reads the 2D window
    [[66,8],[1,64]] at offset (y0+dy)*66 + dx; the padding zeros make every
    tap exact, so there is no edge fixup of any kind.
  - All matmul operands are bf16 (host RNE-rounds fp32 -> bf16): same 1
    cycle/row PE stream rate as fp32r, but half the input DMA bytes and the
    compiler's fast-weight-load (FWL) path for LDWEIGHTS.  PSUM accumulates
    fp32; outputs are stored fp32.  Measured rel err vs f64 ref ~2.5e-3.
  - DMA plan (descriptor-gen is ~0.65us per DMA on the issuing engine, so
    big chunks, split across engines): scalar streams img0 (small first
    chunk to unblock block 0 ASAP), sync streams weights then img1,
    gpsimd memsets the warmup tile.  Output stores are paired (2 blocks =
    4KB/partition per DMA) and split scalar(img0)/sync(img1); the final
    block is split in halves across scalar+sync to shorten the drain.
  - A short bf16 dummy-matmul warmup starts PE activity right after the
    framework preamble (~7us) so the HAM clock gate (3.4us busy window,
    1.2->2.4 GHz) releases before the bulk of the real matmul stream.
"""

import numpy as np
import ml_dtypes

import concourse.bass as bass  # noqa: F401  (registers bass types)
import concourse.tile as tile
import concourse.mybir as mybir
from concourse import bacc, bass_utils

F32 = mybir.dt.float32
BF16 = mybir.dt.bfloat16

B, CIN, COUT, KK, H, W = 16, 128, 128, 3, 64, 64
NCORES = 8
BPC = B // NCORES  # batches per core
HW = H * W         # 4096
PW = W + 2         # padded row length (66)
PH = H + 2         # padded rows (66)
XLEN = PH * PW     # 4356
ROWBLK = 8         # output rows per PSUM block (8*64=512 = one fp32 PSUM bank)
NBLK = H // ROWBLK

TRACE = False      # set True to capture an NTFF profile (fills LAST_EXEC_NS)
LAST_EXEC_NS = None

_CACHE = {}


def _build():
    nc = bacc.Bacc("TRN2", target_bir_lowering=False, debug=False, num_devices=NCORES)
    x_d = nc.dram_tensor("x", [BPC, CIN, XLEN], BF16, kind="ExternalInput").ap()
    w_d = nc.dram_tensor("w", [CIN, KK * KK * COUT], BF16, kind="ExternalInput").ap()
    o_d = nc.dram_tensor("o", [BPC, COUT, HW], F32, kind="ExternalOutput").ap()

    with tile.TileContext(nc) as tc:
        with (
            tc.tile_pool(name="wt", bufs=1) as wtp,
            tc.tile_pool(name="xin", bufs=2) as xp,
            tc.tile_pool(name="ps", bufs=4, space="PSUM") as pp,
            tc.tile_pool(name="ot", bufs=5) as op,
            tc.tile_pool(name="warm", bufs=1) as wmp,
            tc.tile_pool(name="warmps", bufs=1, space="PSUM") as wpp,
        ):
            wt = wtp.tile([CIN, KK * KK * COUT], BF16)
            xins = [
                xp.tile([CIN, XLEN], BF16, tag="xin", name=f"xin{i}")
                for i in range(BPC)
            ]

            # --- input DMAs.  A 128-partition load costs one packet per
            # partition on its queue (~10-20ns each after a ~1.6us
            # desc-gen+ring latency), and rates are per-queue — so the two
            # head-critical pieces (all 9 weight taps; img0 rows 0-9) go
            # first on the two fast HWDGE queues (sync / scalar), with later
            # chunks sized so each lands just before its consumer block.
            # The gpsimd software-DGE queue is slow to start (~+1us), so it
            # only carries img1 (needed from ~25us).
            # A DMA's 16 completion-sem increments only drain after the data
            # packets queued behind it on the same ring, so each queue gets
            # at most two back-to-back input loads and every consumption
            # deadline keeps >=0.8us of margin.
            # sync: w taps 0-5 (gates the first matmul) + img0 rows 10-21
            # (block 1); then sync only carries output stores.
            nc.sync.dma_start(wt[:, : 6 * COUT], w_d[:, : 6 * COUT])
            nc.sync.dma_start(xins[0][:, PW * 10 : PW * 22], x_d[0][:, PW * 10 : PW * 22])
            # scalar: img0 rows 0-9 (block 0, gates the first matmul), img0
            # rows 34-65 (blocks 4-7, ~19us), img1 rows 0-33 (~27us).
            nc.scalar.dma_start(xins[0][:, : PW * 10], x_d[0][:, : PW * 10])
            nc.scalar.dma_start(xins[0][:, PW * 34 :], x_d[0][:, PW * 34 :])
            nc.scalar.dma_start(xins[1][:, : PW * 34], x_d[1][:, : PW * 34])
            # gpsimd (slow-start software-DGE ring): w taps 6-8 (first-MM
            # + 1.3us), img0 rows 22-33 (block 2, ~15us), img1 rows 34-65.
            nc.gpsimd.dma_start(wt[:, 6 * COUT :], w_d[:, 6 * COUT :])
            nc.gpsimd.dma_start(xins[0][:, PW * 22 : PW * 34], x_d[0][:, PW * 22 : PW * 34])
            nc.gpsimd.dma_start(xins[1][:, PW * 34 :], x_d[1][:, PW * 34 :])

            # --- PE warm-up: HAM releases the clock gate only after ~3.4us of
            # sustained array activity; dummy bf16 matmuls on a zeroed tile
            # bridge from the framework preamble (~7.5us) to first data
            # (~10.3us) so most of the real stream runs at 2.4 GHz.
            wz = wmp.tile([CIN, 4 * COUT], BF16)
            nc.vector.memset(wz[:], 0.0)
            wps = wpp.tile([COUT, 4 * COUT], F32)
            for _ in range(6):
                nc.tensor.matmul(wps[:], wz[:, :COUT], wz[:], start=True, stop=True)

            # --- main conv: per image, per 8-row block, 9 accumulating MMs.
            for lb in range(BPC):
                xrf = xins[lb][:].rearrange("p (r c) -> p r c", c=PW)  # [128,66,66]
                ot = None
                for yb in range(NBLK):
                    y0 = yb * ROWBLK
                    ps = pp.tile([COUT, ROWBLK * W], F32)
                    first = True
                    for dy in range(KK):
                        for dx in range(KK):
                            nc.tensor.matmul(
                                ps[:],
                                wt[:, (dy * KK + dx) * COUT : (dy * KK + dx + 1) * COUT],
                                xrf[:, y0 + dy : y0 + dy + ROWBLK, dx : dx + W],
                                start=first,
                                stop=(dy == KK - 1 and dx == KK - 1),
                            )
                            first = False
                    # all paired stores on sync (idle after the two weight
                    # loads; DRAM writes are cheap when the ring is exclusive).
                    eng = nc.sync
                    final_img = lb == BPC - 1
                    if final_img and yb == NBLK - 1:
                        # final block: two independent copy->store chains with
                        # no cross-engine FIFO coupling — vector copy feeds a
                        # sync store, scalar copy feeds scalar's own store.
                        ot = op.tile([COUT, ROWBLK * W], F32, name="ot_last")
                        hw2 = ROWBLK * W // 2
                        nc.vector.tensor_copy(ot[:, :hw2], ps[:, :hw2])
                        nc.sync.dma_start(
                            o_d[lb][:, W * y0 : W * y0 + hw2], ot[:, :hw2]
                        )
                        nc.scalar.copy(ot[:, hw2:], ps[:, hw2:])
                        nc.scalar.dma_start(
                            o_d[lb][:, W * y0 + hw2 : W * y0 + 2 * hw2],
                            ot[:, hw2:],
                        )
                    elif final_img and yb == NBLK - 2:
                        # penultimate block of the last image: single store so
                        # the final block isn't held behind a pair.
                        ot = op.tile([COUT, ROWBLK * W], F32, name="ot_single")
                        nc.vector.tensor_copy(ot[:], ps[:])
                        eng.dma_start(o_d[lb][:, W * y0 : W * (y0 + ROWBLK)], ot[:])
                    elif yb % 2 == 0:
                        # even block: stage into the low half of a paired tile
                        ot = op.tile([COUT, 2 * ROWBLK * W], F32, name="ot_pair")
                        nc.vector.tensor_copy(ot[:, : ROWBLK * W], ps[:])
                    else:
                        # odd block: high half, then one 4KB/partition store
                        nc.vector.tensor_copy(ot[:, ROWBLK * W :], ps[:])
                        eng.dma_start(
                            o_d[lb][:, W * (y0 - ROWBLK) : W * (y0 + ROWBLK)], ot[:]
                        )
    nc.compile()
    return nc


def _get_nc():
    if "nc" not in _CACHE:
        _CACHE["nc"] = _build()
    return _CACHE["nc"]


def kernel(x, weights):
    """x: [16,128,64,64] f32; weights: [128,128,9] f32 -> [2048,64,64] f32."""
    global LAST_EXEC_NS
    x = np.asarray(x, dtype=np.float32)
    w = np.asarray(weights, dtype=np.float32)
    # [cout, cin, k] -> [cin, k, cout] so tap k is a contiguous lhsT slice
    wT = np.ascontiguousarray(w.transpose(1, 2, 0)).reshape(CIN, KK * KK * COUT)
    wT = wT.astype(ml_dtypes.bfloat16)  # RNE
    xpad = np.zeros((B, CIN, PH, PW), ml_dtypes.bfloat16)
    xpad[:, :, 1 : H + 1, 1 : W + 1] = x.astype(ml_dtypes.bfloat16)
    xpad = xpad.reshape(B, CIN, XLEN)

    nc = _get_nc()
    xr = xpad.reshape(NCORES, BPC, CIN, XLEN)
    in_maps = [{"x": np.ascontiguousarray(xr[c]), "w": wT} for c in range(NCORES)]

    res = bass_utils.run_bass_kernel_spmd(
        nc, in_maps, core_ids=list(range(NCORES)), trace=TRACE
    )
    LAST_EXEC_NS = res.exec_time_ns

    arr = np.stack([res.results[c]["o"] for c in range(NCORES)])  # [8, 2, 128, 4096]
    # out[cout*B + b] = conv[b, cout], with b = core*BPC + lb
    arr = arr.transpose(2, 0, 1, 3).reshape(COUT, B, H, W)
    return np.ascontiguousarray(arr.reshape(COUT * B, H, W))
